# revision 28
# baseline (speedup 1.0000x reference)
"""Trainium2 Bass kernel for a masked transformer block + classifier head.

Sharding: data-parallel over batch across 8 NeuronCores; each core runs the
full block for one batch element (no collectives).

Precision plan (tolerance 2e-2; measured ~7.6e-3):
  - x / Wq / Wk / Wv / xn in bf16 (halves the serial DMA bytes gating the
    first scores); scores accumulate in fp32 PSUM and Q/K land as f32r,
  - exp probabilities eT in fp8e5m2; V, attn, Wo, W1, xn2 in fp8e4m3 with
    host-side power-of-2 scale folding (Wv*8, Wo*4, W1*32) so every fp8
    tensor sits in the format's normal range; the scales cancel exactly: the
    residual stream is carried as 32*x2 (LayerNorm is scale-invariant, eps
    compensated; the mean-pool folds 1/(32N) into the pooling vector).
  - AV / Wo / FFN1 run as fp8 DoubleRow (2 K-tiles per pass, 0.5 cycles/row
    = 4x the bf16 K-throughput).
  - W2 is applied ONLY to the token-mean of the gelu activations: the model
    ends in a mean-pool, so mean(x3) = mean(x2) + mean(g) @ W2 + b2; mean(g)
    rides the gelu's accum_out for free and W2 stays bf16, killing both the
    whole per-token FFN2 stream and the dominant fp8 weight-quantization
    error (weight rounding is systematic across tokens and survives the
    pool; activation rounding averages out).
  - LayerNorm gains/biases are host-folded into the downstream weight/bias
    (W' = g (.) W, b' = b_lin + b_ln @ W), so the device applies LN as a pure
    (x-mu)*rstd normalize and every transpose drains PSUM->SBUF with a single
    plain copy per token chunk.
  - residual adds ride the PE: the Wo PSUM accumulation is pre-loaded with
    the bf16 residual via an identity matmul; PSUM->SBUF drains split
    across ACT/DVE.

Attention per head: scoresT = k @ q^T per key chunk ([128 keys, 1024 tok]),
exp on ACT (1/8 scale fused) straight from PSUM, then AV as DoubleRow with
lhsT = [V | 1] so the softmax denominator lands at PSUM partition 64; its
reciprocal is written partition-64-aligned into SBUF and broadcast across
the 64 output partitions with a [1,64]-ones matmul. No DMA anywhere in the
attention loop. Engines execute in order, so emission order is the
schedule: V and the Q/K chunks for heads 4-7 are woven between the score
chunks of heads 1-3 so the ACT exp stream (the phase bottleneck: 64 exps of
[128,1024] ~ 66us is a hard floor, softmax over 8.4M scores at 1 col/cycle)
never starves.
"""

import sys

sys.path.insert(0, '/opt/trn_rl_repo')

from contextlib import ExitStack

import numpy as np

import concourse.bass as bass
import concourse.mybir as mybir
import concourse.tile as tile
from concourse import bacc
from concourse.bass_utils import run_bass_kernel_spmd
from concourse.masks import make_identity

P = 128
N = 1024        # tokens
D = 512         # model dim
F = 2048        # mlp dim
C = 1000        # classes
H = 8           # heads
DH = 64         # head dim
HB = DH + 1     # head AV block: 64 V dims + a denominator ones column
HBP = DH + 2    # padded V block stride: DoubleRow k-tile step must be %16==0
                # (8 heads * 66 = 528 bytes)
NT = N // P     # 8 token chunks
DC = D // P     # 4 model-dim chunks
FC = F // P     # 16 mlp chunks
SCALE = DH ** -0.5
EPS = 1e-5
RS = 32.0       # residual-stream scale (x2s = RS * x2)
VS = 8.0        # V scale (host-folded into Wv/bv)
N_CORES = 8

F32 = mybir.dt.float32
F32R = mybir.dt.float32r
BF16 = mybir.dt.bfloat16
F8E4 = mybir.dt.float8e4
F8E5 = mybir.dt.float8e5
AF = mybir.ActivationFunctionType
ALU = mybir.AluOpType
DR = mybir.MatmulPerfMode.DoubleRow


def _pin_exp_ln_table_set(arch: str):
    """Make Exp and Ln resolve only to the combined natural_log_exp set.

    bacc's table-load inserter greedily picks the first act-table set
    containing each function; Exp alone would pick exp_and_others and Ln
    would pick natural_log, thrashing ~1.3us table loads on every LN<->attn
    alternation. get_activation_tables() is functools.cache'd, so mutating
    the cached dict in place (set IDs = dict order are preserved) pins both
    functions to the one set that holds them together."""
    from concourse.hw_specs import get_activation_tables
    tables = get_activation_tables(arch)
    for name, funcs in tables.items():
        if name == 'natural_log_exp_and_others':
            continue
        funcs.discard(AF.Exp)
        funcs.discard(AF.Ln)


def build_bass():
    nc = bacc.Bacc(None, target_bir_lowering=False)
    _pin_exp_ln_table_set(nc.m.arch)

    x_d = nc.dram_tensor('x', [N, D], BF16, kind='ExternalInput')
    mask_d = nc.dram_tensor('mask', [N, 1], F32, kind='ExternalInput')
    vec_d = {}
    for nm, sz in [('ln1_g', D), ('ln1_b', D), ('bq', D), ('bk', D), ('bv', D),
                   ('bo', D), ('ln2_g', D), ('ln2_b', D), ('b1', F), ('b2', D),
                   ('lnh_g', D), ('lnh_b', D), ('bh', C)]:
        vec_d[nm] = nc.dram_tensor(nm, [sz], F32, kind='ExternalInput')
    # Weights pre-scaled/folded on the host (see make_in_maps).
    w_d = {nm: nc.dram_tensor(nm, shp, dt, kind='ExternalInput')
           for nm, shp, dt in [('Wq', [D, D], BF16), ('Wk', [D, D], BF16),
                               ('Wv', [D, D], BF16), ('Wo', [D, D], F8E4),
                               ('W1', [D, F], F8E4), ('W2', [F, D], BF16),
                               ('Wh', [D, C], BF16)]}
    out_d = nc.dram_tensor('out', [1, C], F32, kind='ExternalOutput')

    with tile.TileContext(nc) as tc, ExitStack() as top:
        consts = top.enter_context(tc.tile_pool(name='consts', bufs=1))
        wts = top.enter_context(tc.tile_pool(name='wts', bufs=1))
        acts = top.enter_context(tc.tile_pool(name='acts', bufs=1))
        mvp = top.enter_context(tc.tile_pool(name='mv', bufs=4))
        zp = top.enter_context(tc.tile_pool(name='z', bufs=2))

        # ---------------- inputs first: x gates LN1, the critical-path start
        maskT = consts.tile([P, NT], F32)
        nc.sync.dma_start(out=maskT,
                          in_=mask_d[:].rearrange('(c p) o -> p (c o)', p=P))
        x_res = [acts.tile([P, D], BF16, name=f'xres{i}') for i in range(NT)]
        for i in range(NT):
            nc.sync.dma_start(out=x_res[i], in_=x_d[i * P:(i + 1) * P, :])

        vec_pm = {}   # [D]-vectors partition-major: [128, DC]
        for nm in ['bq', 'bk']:
            t = consts.tile([P, DC], F32, name=f'v_{nm}')
            nc.sync.dma_start(out=t,
                              in_=vec_d[nm][:].rearrange('(c p) -> p c', p=P))
            vec_pm[nm] = t

        # -- weights: direct DMA in order of first use (bf16 q/k/v halve the
        # serial DMA bytes gating the first score matmuls) --
        wq_r = wts.tile([P, DC, D], BF16)
        wk_r = wts.tile([P, DC, D], BF16)
        wv_r = wts.tile([P, DC, D], BF16)
        for dst, nm in [(wq_r, 'Wq'), (wk_r, 'Wk'), (wv_r, 'Wv')]:
            nc.sync.dma_start(
                out=dst, in_=w_d[nm][:].rearrange('(c p) m -> p c m', p=P))
        bv_bc = consts.tile([P, D], F32, name='bv_bc')
        nc.sync.dma_start(out=bv_bc,
                          in_=bass.AP(vec_d['bv'], 0, [[0, P], [1, D]]))
        bo_bc = consts.tile([P, D], F32, name='bo_bc')
        nc.sync.dma_start(out=bo_bc, in_=bass.AP(vec_d['bo'], 0, [[0, P], [1, D]]))
        b1T = consts.tile([P, FC], F32)
        nc.sync.dma_start(out=b1T,
                          in_=vec_d['b1'][:].rearrange('(c p) -> p c', p=P))
        wo_f8 = wts.tile([P, 2, 2, D], F8E4)
        nc.sync.dma_start(
            out=wo_f8,
            in_=w_d['Wo'][:].rearrange('(kp ki p) m -> p kp ki m', kp=2, ki=2))
        w1_f8 = wts.tile([P, 2, 2, F], F8E4)
        nc.sync.dma_start(
            out=w1_f8,
            in_=w_d['W1'][:].rearrange('(kp ki p) m -> p kp ki m', kp=2, ki=2))
        # W2 is only ever applied to the token-MEAN of the gelu activations
        # (the model ends in a mean-pool; per-token FFN2 output is never
        # needed): host ships W2/N in bf16 for the pooled accumulation.
        w2p_bf = wts.tile([P, FC, D], BF16)
        nc.sync.dma_start(
            out=w2p_bf, in_=w_d['W2'][:].rearrange('(c p) m -> p c m', p=P))
        wh_bf = wts.tile([P, DC, C], BF16)
        nc.sync.dma_start(
            out=wh_bf, in_=w_d['Wh'][:].rearrange('(c p) m -> p c m', p=P))

        # ---------------- constants ----------------
        eps_sb = consts.tile([P, 1], F32)
        nc.vector.memset(eps_sb, EPS)
        eps2_sb = consts.tile([P, 1], F32)
        nc.vector.memset(eps2_sb, EPS * RS * RS)   # LN2 runs on RS-scaled x2
        identf = consts.tile([P, P], F32)
        make_identity(nc, identf)
        ident_bf = consts.tile([P, P], BF16)
        nc.vector.tensor_copy(ident_bf, identf)
        ones_bf = consts.tile([P, 1], BF16)
        nc.vector.memset(ones_bf, 1.0 / (RS * N))  # pooling vector: mean/RS
        one1_bf = consts.tile([1, 1], BF16)
        nc.vector.memset(one1_bf, 1.0)
        msqT = consts.tile([P, NT], F32)
        nc.vector.tensor_tensor(out=msqT, in0=maskT, in1=maskT, op=ALU.mult)
        mask32T = consts.tile([P, NT], F32)
        nc.gpsimd.tensor_scalar_mul(mask32T, maskT, RS)
        bo32_bc = consts.tile([P, D], F32)
        nc.gpsimd.tensor_scalar_mul(bo32_bc, bo_bc, RS)
        # softmax-denominator broadcast helpers (all partition-64 aligned)
        ones_r = consts.tile([DH + 1, DH], F32R)
        nc.vector.memset(ones_r[DH:DH + 1, :].bitcast(F32), 1.0)
        recip_sb = consts.tile([DH + 1, 4, 512], F32R)

        # long-lived activations
        xnT = acts.tile([P, DC, N], BF16, name='xnT')
        QT = [acts.tile([P, N], F32R, name=f'QT{j}') for j in range(DC)]
        KT = [acts.tile([P, N], F32R, name=f'KT{j}') for j in range(DC)]
        # V in fp8: [mp, parity, head*HB], col 64 of each head block is ones
        # so AV also accumulates the softmax denominator at PSUM row 64
        vp = acts.tile([P, NT // 2, 2, H * HBP], F8E4, name='vp')
        for h in range(H):
            nc.gpsimd.memset(vp[:, :, :, h * HBP + DH:h * HBP + DH + 1], 1.0)
        attnT = [acts.tile([P, 2, N], F8E4, name=f'attnT{kp}') for kp in range(2)]
        # residual stream RS*(m*x)+RS*bo in bf16: PSUM-preloaded into the Wo
        # accumulation via an identity matmul so no vector engine adds it
        res_bf = [acts.tile([P, D], BF16, name=f'resbf{i}') for i in range(NT)]
        x2 = [acts.tile([P, D], BF16, name=f'x2_{i}') for i in range(NT)]
        xn2T = acts.tile([P, 2, 2, N], F8E4, name='xn2T')
        gsumT = acts.tile([P, FC], BF16, name='gsumT')

        # ------------- phase B: mask + LN1 + transpose -------------
        def ln1_chunk(i):
            # LN1 stats on the RAW x chunk; the mask folds in as
            # var' = m^2 var (Ln scale AP) and z = (x-mu)*(m*rstd'),
            # keeping the mask multiply off the critical chain.
            mv6 = mvp.tile([P, 6], F32, tag='mv6', name='mv6b')
            mv2 = mvp.tile([P, 2], F32, tag='mv2', name='mv2b')
            nc.vector.bn_stats(out=mv6, in_=x_res[i])
            nc.vector.bn_aggr(out=mv2, in_=mv6)
            nc.scalar.activation(out=mv2[:, 1:2], in_=mv2[:, 1:2],
                                 func=AF.Ln, bias=eps_sb,
                                 scale=msqT[:, i:i + 1])
            nc.scalar.activation(out=mv2[:, 1:2], in_=mv2[:, 1:2],
                                 func=AF.Exp, scale=-0.5)
            nc.gpsimd.tensor_scalar_mul(mv2[:, 1:2], mv2[:, 1:2],
                                        maskT[:, i:i + 1])
            z = zp.tile([P, D], BF16, tag='z', name='zb')
            nc.gpsimd.tensor_scalar(out=z, in0=x_res[i],
                                    scalar1=mv2[:, 0:1],
                                    scalar2=mv2[:, 1:2],
                                    op0=ALU.subtract, op1=ALU.mult)
            # pav/pb slots are idle until attention starts; LN1's transpose
            # staging borrows them so it never contends with the qk psum
            pt = ps_cd.tile([P, D], BF16, tag=('pav' if i % 2 == 0 else 'pb'),
                            bufs=1, name='ptB')
            for j in range(DC):
                nc.tensor.transpose(pt[:, j * P:(j + 1) * P],
                                    z[:, j * P:(j + 1) * P], ident_bf)
            nc.vector.tensor_copy(xnT[:, :, i * P:(i + 1) * P], pt)

        # ------------- phases C+D: QKV interleaved with attention -------------
        # Engines execute their queues IN ORDER, so emission order is the PE
        # schedule: Q/K for heads 0-3 first, head-0 scores, then V and the
        # remaining Q/K chunks woven between later score chunks so the ACT
        # exp stream never starves and no PE instruction waits on an
        # unfinished producer.
        es_ps = ExitStack()
        ps_cd = es_ps.enter_context(tc.tile_pool(name='ps_cd', bufs=2,
                                                 space='PSUM'))

        def qk_unit(j, di, nh):
            # one (dst, nh) quarter of a Q/K projection chunk
            dst, w_r, bT = [(QT, wq_r, vec_pm['bq']),
                            (KT, wk_r, vec_pm['bk'])][di]
            pm = ps_cd.tile([P, 512], F32, tag='qv', bufs=2, name='pmC')
            for kc in range(DC):
                nc.tensor.matmul(
                    pm, w_r[:, kc, j * P:(j + 1) * P],
                    xnT[:, kc, nh * 512:(nh + 1) * 512],
                    start=(kc == 0), stop=(kc == DC - 1))
            if j == 0 and di == 0:
                nc.scalar.activation(
                    out=dst[j][:, nh * 512:(nh + 1) * 512], in_=pm,
                    func=AF.Identity, bias=bT[:, j:j + 1], scale=1.0)
            else:
                nc.any.tensor_scalar_add(
                    out=dst[j][:, nh * 512:(nh + 1) * 512], in0=pm,
                    scalar1=bT[:, j:j + 1])

        def v_unit(mp, par):
            i = 2 * mp + par
            pm = ps_cd.tile([P, 512], F32, tag='qv', bufs=2, name='pmV')
            for kc in range(DC):
                nc.tensor.matmul(pm, xnT[:, kc, i * P:(i + 1) * P],
                                 wv_r[:, kc, :],
                                 start=(kc == 0), stop=(kc == DC - 1))
            vrow = vp[:, mp, par, :].rearrange('p (h c) -> p h c', h=H)
            nc.vector.tensor_tensor(
                out=vrow[:, :, 0:DH],
                in0=pm[:].rearrange('p (h c) -> p h c', h=H),
                in1=bv_bc[:].rearrange('p (h c) -> p h c', h=H),
                op=ALU.add)

        eT_all = {}

        def score_unit(et_pool, h, mp, par):
            p0 = DH * (h % 2)
            hj = h // 2
            if par == 0:
                eT_all[(h, mp)] = et_pool.tile([P, 2, N], F8E5, tag=f'e{mp}',
                                               name=f'eT{mp}')
            m = 2 * mp + par
            pss = ps_cd.tile([P, N], F32, tag='pss', bufs=2, name='pss')
            for nh in range(2):
                nc.tensor.matmul(
                    pss[:, nh * 512:(nh + 1) * 512],
                    KT[hj][p0:p0 + DH, m * P:(m + 1) * P],
                    QT[hj][p0:p0 + DH, nh * 512:(nh + 1) * 512],
                    start=True, stop=True)
            nc.scalar.activation(out=eT_all[(h, mp)][:, par, :], in_=pss,
                                 func=AF.Exp, scale=SCALE)

        def av_head(h, pool, astg, tags=('pav', 'pb'), bufs=(1, 1)):
            hj = h // 2
            p0 = DH * (h % 2)
            for nh in range(2):
                pav = pool.tile([HB, 512], F32, tag=tags[0], bufs=bufs[0],
                                name='pav')
                for mp in range(NT // 2):
                    nc.tensor.matmul(
                        pav, vp[:, mp, :, h * HBP:h * HBP + HB],
                        eT_all[(h, mp)][:, :, nh * 512:(nh + 1) * 512],
                        start=(mp == 0), stop=(mp == NT // 2 - 1),
                        perf_mode=DR)
                # fused reciprocal straight from the PSUM denominator row
                # (partition 64 on both sides: quadrant-aligned)
                slot = (h % 2) * 2 + nh
                with nc.allow_low_precision(reason='softmax denom f32r'):
                    nc.vector.reciprocal(
                        out=recip_sb[DH:DH + 1, slot, :],
                        in_=pav[DH:DH + 1, :])
                # engines read at most one PSUM operand per instruction, so
                # stage the unnormalized AV rows in SBUF (this also frees the
                # pav bank early); the broadcast matmul result stays in PSUM
                a_bf = astg.tile([DH, 512], BF16, tag=f'a{nh}', name='a_bf')
                if nh == 0:
                    nc.vector.tensor_copy(a_bf, pav[0:DH, :])
                else:
                    nc.scalar.activation(out=a_bf, in_=pav[0:DH, :],
                                         func=AF.Copy, scale=1.0)
                # broadcast 1/rowsum across 64 partitions and normalize
                # into fp8 attnT (x8 head-scale folded into Wv/bv)
                pb = pool.tile([DH, 512], F32, tag=tags[1], bufs=bufs[1],
                               name='pbn')
                nc.tensor.matmul(pb, ones_r[DH:DH + 1, :],
                                 recip_sb[DH:DH + 1, slot, :],
                                 start=True, stop=True)
                nc.vector.tensor_tensor(
                    out=attnT[hj // 2][p0:p0 + DH, hj % 2,
                                       nh * 512:(nh + 1) * 512],
                    in0=a_bf, in1=pb, op=ALU.mult)

        es_pse = ExitStack()
        with tc.tile_pool(name='et', bufs=2) as et_pool, \
             tc.tile_pool(name='astg', bufs=2) as astg:
            fill = []                       # PE filler units, emitted between
            for di in range(2):             # score chunks of heads 0-2
                for nh in range(2):
                    fill.append(lambda di=di, nh=nh: qk_unit(1, di, nh))
            for mp in range(NT // 2):
                for par in range(2):
                    fill.append(lambda mp=mp, par=par: v_unit(mp, par))
            for j in (2, 3):
                for di in range(2):
                    for nh in range(2):
                        fill.append(lambda j=j, di=di, nh=nh: qk_unit(j, di, nh))

            # LN1 + the head-0 Q/K chunks, interleaved so the nh0 projection
            # halves (which only need token chunks 0-3) run during the LN1
            # tail and the first exp fires right after LN1 chunk 7
            for i in range(4):
                ln1_chunk(i)
            qk_unit(0, 0, 0)
            qk_unit(0, 1, 0)
            for i in range(4, NT):
                ln1_chunk(i)
            qk_unit(0, 0, 1)
            qk_unit(0, 1, 1)
            # residual = RS*(m*x) + RS*bo on gpsimd, deferred out of the LN1
            # window (first consumer is the Wo PSUM preload at ~95us)
            for i in range(NT):
                nc.gpsimd.tensor_scalar_mul(res_bf[i], x_res[i],
                                            mask32T[:, i:i + 1])
                nc.gpsimd.tensor_tensor(out=res_bf[i], in0=res_bf[i],
                                        in1=bo32_bc, op=ALU.add)
            for h in range(H):
                for mp in range(NT // 2):
                    for par in range(2):
                        score_unit(et_pool, h, mp, par)
                        if h in (0, 1, 2) and fill:
                            fill.pop(0)()
                        # AV for the previous head as soon as its inputs
                        # exist (for h-1 == 0 that includes all of V, whose
                        # units finish during h == 1)
                        if h >= 2 and (mp, par) == (0, 1):
                            av_head(h - 1, ps_cd, astg)
                if h == 1:
                    av_head(0, ps_cd, astg)  # all 8 V units just emitted
            # last head's AV runs in the phase-E pool scope: more free PSUM
            # slots there let its two nh chains overlap instead of
            # serializing on a single pav/pb slot. ps_cd must close first
            # (its tags still hold all 8 PSUM banks).
            es_ps.close()
            ps_e = es_pse.enter_context(tc.tile_pool(name='ps_e', bufs=3,
                                                     space='PSUM'))
            av_head(H - 1, ps_e, astg, tags=('wo', 'wo'), bufs=(3, 3))

        # ------------- phase E: Wo + LN2 -------------
        for i in range(NT):
            pm = ps_e.tile([P, D], F32, tag='wo', bufs=3, name='pmWo')
            nc.tensor.matmul(pm, ident_bf, res_bf[i], start=True, stop=False,
                             skip_group_check=True)
            for kp in range(2):
                nc.tensor.matmul(pm, attnT[kp][:, :, i * P:(i + 1) * P],
                                 wo_f8[:, kp], start=False, stop=(kp == 1),
                                 perf_mode=DR, skip_group_check=True)
            if i % 2 == 0:
                nc.scalar.activation(out=x2[i], in_=pm, func=AF.Copy,
                                     scale=1.0)
            else:
                nc.vector.tensor_copy(x2[i], pm)
            # LN2 on the RS-scaled residual stream (scale-invariant, eps
            # compensated via eps2); stats read the PSUM directly so they
            # don't serialize behind the x2 drain
            mv6 = mvp.tile([P, 6], F32, tag='mv6', name='mv6e')
            mv2 = mvp.tile([P, 2], F32, tag='mv2', name='mv2e')
            nc.vector.bn_stats(out=mv6, in_=x2[i])
            nc.vector.bn_aggr(out=mv2, in_=mv6)
            nc.scalar.activation(out=mv2[:, 1:2], in_=mv2[:, 1:2],
                                 func=AF.Ln, bias=eps2_sb, scale=1.0)
            nc.scalar.activation(out=mv2[:, 1:2], in_=mv2[:, 1:2],
                                 func=AF.Exp, scale=-0.5)
            z = zp.tile([P, D], BF16, tag='z', name='ze')
            nc.gpsimd.tensor_scalar(out=z, in0=x2[i],
                                    scalar1=mv2[:, 0:1],
                                    scalar2=mv2[:, 1:2],
                                    op0=ALU.subtract, op1=ALU.mult)
            pt = ps_e.tile([P, D], BF16, tag='pt2', bufs=2, name='ptE')
            for j in range(DC):
                nc.tensor.transpose(pt[:, j * P:(j + 1) * P],
                                    z[:, j * P:(j + 1) * P], ident_bf)
            if i % 2 == 0:
                nc.vector.tensor_copy(xn2T[:, :, :, i * P:(i + 1) * P], pt)
            else:
                nc.scalar.activation(out=xn2T[:, :, :, i * P:(i + 1) * P],
                                     in_=pt, func=AF.Copy, scale=1.0)

        # gelu table hoist: a dummy 1-element gelu right after the last LN2
        # transcendental makes the ACT table swap happen here, off the
        # first-real-gelu critical path (the remaining phase-E ACT ops are
        # Copy/Identity, present in every table set)
        gd = zp.tile([P, 1], F32, tag='gd', name='gdummy')
        nc.scalar.activation(out=gd[0:1], in_=eps_sb[0:1],
                             func=AF.Gelu_apprx_tanh, bias=eps_sb[0:1],
                             scale=1.0)

        # ------------- phase F: FFN + pool + head -------------
        es_pse.close()
        es_psf = ExitStack()
        ps_f = es_psf.enter_context(tc.tile_pool(name='ps_f', bufs=1,
                                                 space='PSUM'))
        with tc.tile_pool(name='p_f', bufs=1) as p_f:
            b2_bf = p_f.tile([1, D], BF16)
            nc.gpsimd.dma_start(out=b2_bf,
                                in_=bass.AP(vec_d['b2'], 0, [[0, 1], [1, D]]))
            bh_sb = p_f.tile([1, C], F32)
            nc.sync.dma_start(out=bh_sb,
                              in_=bass.AP(vec_d['bh'], 0, [[0, 1], [1, C]]))
            # pool accumulator group: preload b2, add mean(x2) as soon as
            # x2 exists (PE is idle in phase E/F start), then stream in the
            # pooled-FFN contributions mean(g) @ W2/N as each gelu's
            # accum_out lands. Per-token FFN2 output is never materialized.
            pp = ps_f.tile([1, D], F32, tag='sm', bufs=1, name='pp')
            nc.tensor.matmul(pp, one1_bf, b2_bf, start=True, stop=False,
                             skip_group_check=True)
            for i in range(NT):
                nc.tensor.matmul(pp, ones_bf, x2[i], start=False, stop=False,
                                 skip_group_check=True)
            with tc.tile_pool(name='gscr', bufs=2) as gscr_pool:
                for fc in range(FC):
                    pm = ps_f.tile([P, N], F32, tag='f1', bufs=2, name='pmF1')
                    for nh in range(2):
                        for kp in range(2):
                            nc.tensor.matmul(
                                pm[:, nh * 512:(nh + 1) * 512],
                                w1_f8[:, kp, :, fc * P:(fc + 1) * P],
                                xn2T[:, kp, :, nh * 512:(nh + 1) * 512],
                                start=(kp == 0), stop=(kp == 1), perf_mode=DR)
                    gscr = gscr_pool.tile([P, N], F8E4, tag='g', name='gscr')
                    with nc.allow_low_precision(reason='gelu token-sum bf16'):
                        nc.scalar.activation(
                            out=gscr, in_=pm,
                            func=AF.Gelu_apprx_tanh, bias=b1T[:, fc:fc + 1],
                            scale=1.0 / RS, accum_out=gsumT[:, fc:fc + 1])
                    if fc > 0:
                        nc.tensor.matmul(pp, gsumT[:, fc - 1:fc],
                                         w2p_bf[:, fc - 1, :], start=False,
                                         stop=False, skip_group_check=True)
                nc.tensor.matmul(pp, gsumT[:, FC - 1:FC],
                                 w2p_bf[:, FC - 1, :], start=False,
                                 stop=True, skip_group_check=True)
            # head layernorm directly on the pooled PSUM vector
            mv6 = mvp.tile([P, 6], F32, tag='mv6', name='mv6h')
            mv2 = mvp.tile([P, 2], F32, tag='mv2', name='mv2h')
            nc.vector.bn_stats(out=mv6[0:1], in_=pp)
            nc.vector.bn_aggr(out=mv2[0:1], in_=mv6[0:1])
            nc.scalar.activation(out=mv2[0:1, 1:2], in_=mv2[0:1, 1:2],
                                 func=AF.Ln, bias=eps_sb[0:1], scale=1.0)
            nc.scalar.activation(out=mv2[0:1, 1:2], in_=mv2[0:1, 1:2],
                                 func=AF.Exp, scale=-0.5)
            zh = zp.tile([P, D], F32, tag='z', name='zh')
            nc.vector.tensor_scalar(out=zh[0:1], in0=pp,
                                    scalar1=mv2[0:1, 0:1],
                                    scalar2=mv2[0:1, 1:2],
                                    op0=ALU.subtract, op1=ALU.mult)
            zT_r = acts.tile([P, DC], BF16, tag='zT')
            pth = ps_f.tile([P, DC], F32, tag='sm2', bufs=1, name='pth')
            for j in range(DC):
                nc.tensor.transpose(pth[:, j:j + 1],
                                    zh[0:1, j * P:(j + 1) * P],
                                    identf[0:1, 0:1])
            nc.vector.tensor_copy(zT_r, pth)
            out_sb = p_f.tile([1, C], F32, tag='osb')
            for half in range(2):
                ph = ps_f.tile([1, 500], F32, tag='sm2', bufs=1, name='ph')
                for j in range(DC):
                    nc.tensor.matmul(
                        ph, zT_r[:, j:j + 1],
                        wh_bf[:, j, half * 500:(half + 1) * 500],
                        start=(j == 0), stop=(j == DC - 1))
                nc.vector.tensor_tensor(
                    out=out_sb[:, half * 500:(half + 1) * 500], in0=ph,
                    in1=bh_sb[:, half * 500:(half + 1) * 500], op=ALU.add)
                nc.sync.dma_start(
                    out=out_d[:, half * 500:(half + 1) * 500],
                    in_=out_sb[:, half * 500:(half + 1) * 500])
        es_psf.close()

    nc.finalize()
    return nc


_NC_CACHE = None


def make_in_maps(inputs):
    import ml_dtypes
    f8 = ml_dtypes.float8_e4m3
    arr = {k: np.asarray(v, dtype=np.float32) for k, v in inputs.items()}
    # Fold LayerNorm gains/biases into the downstream weights/biases
    # (mathematically exact: (z*g+b) @ W + c == z @ (g(.)W) + (b@W + c)),
    # and apply exact power-of-two scales so fp8 tensors sit in range.
    g1, b1n = arr['ln1_g'][:, None], arr['ln1_b']
    arr['bq'] = arr['bq'] + b1n @ arr['Wq']
    arr['bk'] = arr['bk'] + b1n @ arr['Wk']
    arr['bv'] = (arr['bv'] + b1n @ arr['Wv']) * VS
    arr['Wq'] = (g1 * arr['Wq']).astype(ml_dtypes.bfloat16)
    arr['Wk'] = (g1 * arr['Wk']).astype(ml_dtypes.bfloat16)
    arr['Wv'] = (g1 * arr['Wv'] * VS).astype(ml_dtypes.bfloat16)
    arr['x'] = arr['x'].astype(ml_dtypes.bfloat16)
    # (VS*attn) @ (RS/VS * Wo) = RS * attn@Wo; bo is RS-scaled on device
    arr['Wo'] = (arr['Wo'] * (RS / VS)).astype(f8)
    g2, b2n = arr['ln2_g'][:, None], arr['ln2_b']
    arr['b1'] = arr['b1'] + b2n @ arr['W1']
    arr['W1'] = (g2 * arr['W1'] * RS).astype(f8)
    arr['W2'] = (arr['W2'] / N).astype(ml_dtypes.bfloat16)
    gh, bhn = arr['lnh_g'][:, None], arr['lnh_b']
    arr['bh'] = arr['bh'] + bhn @ arr['Wh']
    arr['Wh'] = (gh * arr['Wh']).astype(ml_dtypes.bfloat16)
    out = {}
    for k, v in arr.items():
        out[k] = np.ascontiguousarray(v)
    x = out.pop('x')                       # [8, 1024, 512]
    return [dict(out, x=np.ascontiguousarray(x[i])) for i in range(N_CORES)]


def kernel(**inputs) -> np.ndarray:
    global _NC_CACHE
    if _NC_CACHE is None:
        _NC_CACHE = build_bass()
    nc = _NC_CACHE

    in_maps = make_in_maps(inputs)
    res = run_bass_kernel_spmd(nc, in_maps, core_ids=list(range(N_CORES)))
    return np.concatenate([res.results[i]['out'] for i in range(N_CORES)],
                          axis=0)


if __name__ == '__main__':
    rng = np.random.default_rng(0)
    s = lambda d: 1.0 / np.sqrt(d)
    ins = {
        'x': rng.standard_normal((8, N, D), dtype=np.float32),
        'mask': np.ones((N, 1), np.float32),
        'ln1_g': np.ones(D, np.float32), 'ln1_b': np.zeros(D, np.float32),
        'Wq': rng.standard_normal((D, D), dtype=np.float32) * s(D),
        'bq': np.zeros(D, np.float32),
        'Wk': rng.standard_normal((D, D), dtype=np.float32) * s(D),
        'bk': np.zeros(D, np.float32),
        'Wv': rng.standard_normal((D, D), dtype=np.float32) * s(D),
        'bv': np.zeros(D, np.float32),
        'Wo': rng.standard_normal((D, D), dtype=np.float32) * s(D),
        'bo': np.zeros(D, np.float32),
        'ln2_g': np.ones(D, np.float32), 'ln2_b': np.zeros(D, np.float32),
        'W1': rng.standard_normal((D, F), dtype=np.float32) * s(D),
        'b1': np.zeros(F, np.float32),
        'W2': rng.standard_normal((F, D), dtype=np.float32) * s(F),
        'b2': np.zeros(D, np.float32),
        'lnh_g': np.ones(D, np.float32), 'lnh_b': np.zeros(D, np.float32),
        'Wh': rng.standard_normal((D, C), dtype=np.float32) * s(D),
        'bh': np.zeros(C, np.float32),
    }
    out = kernel(**ins)
    print('out', out.shape, out.dtype, float(np.abs(out).max()))
